# revision 10
# baseline (speedup 1.0000x reference)
"""LocallyConnected2D (B=16, H=W=64, C=32, 3x3 valid, F=64) on 8 trn2 cores.

out[b, oh, ow, f] = sum_{kh,kw,c} x[b, oh+kh, ow+kw, c] * kernel[p, (kh,kw,c), f] + bias[p, f]
with p = oh*62+ow.  P=3844 sharded by oh-rows across 8 cores (8 rows/core,
core 7 padded).  Per core, per position: 3 fp32 matmuls (K=97/96/96, N=64)
accumulating in PSUM; patchesT tiles built once per x-row via PE transpose;
bias rides as a 97th contraction row against a constant-ones row in patchesT.
"""

import sys

for _p in ("/opt/trn_rl_repo",):
    if _p not in sys.path:
        sys.path.insert(0, _p)

import numpy as np
from contextlib import ExitStack

import concourse.bass as bass
import concourse.bacc as bacc
import concourse.mybir as mybir
import concourse.tile as tile
from concourse.bass_utils import run_bass_kernel_spmd
from concourse.masks import make_identity

F32 = mybir.dt.float32

B, H, W, C = 16, 64, 64, 32
KH, KW = 3, 3
OH, OW = 62, 62
F = 64
KSZ = KH * KW * C  # 288
KCH = KW * C       # 96 per kh-chunk
NCORES = 8
RPC = 8            # oh rows per core (core 7: 2 rows are padding)
NXR = RPC + 2      # x rows staged per core
PPC = RPC * OW     # 496 positions per core (padded for core 7)

_cached = {}


def _build_program():
    if "nc" in _cached:
        return _cached["nc"]

    nc = bacc.Bacc(None)
    xs = nc.declare_dram_parameter("xs", [B, NXR, W, C], F32, isOutput=False)
    # kernel pre-transposed on host to the SBUF tile layout:
    # ks2[oh*8+kg, kk, i, ch, f] = kernel[p0(oh,kg)+i, ch*96+kk, f]
    ks2 = nc.declare_dram_parameter("ks2", [RPC * 8, KCH, 8, KH, F], F32, isOutput=False)
    bs = nc.declare_dram_parameter("bs", [PPC, F], F32, isOutput=False)
    out = nc.declare_dram_parameter("out", [B, RPC, OW, F], F32, isOutput=True)

    with ExitStack() as ctx:
        tc = ctx.enter_context(tile.TileContext(nc))
        const_pool = ctx.enter_context(tc.tile_pool(name="const", bufs=1))
        tpool = ctx.enter_context(tc.tile_pool(name="tpool", bufs=NXR))
        papool = ctx.enter_context(tc.tile_pool(name="papool", bufs=3))
        tppool = ctx.enter_context(tc.tile_pool(name="tppool", bufs=2, space="PSUM"))
        ktpool = ctx.enter_context(tc.tile_pool(name="ktpool", bufs=6))
        pspool = ctx.enter_context(tc.tile_pool(name="pspool", bufs=6, space="PSUM"))
        stpool = ctx.enter_context(tc.tile_pool(name="stpool", bufs=2))

        identity = const_pool.tile([128, 128], F32)
        make_identity(nc, identity)

        # --- patchesT tiles: T[r][kw*32+c, ow*16+b] = x[b, r, ow+kw, c]; row 96 = 1.0
        T = []
        for r in range(NXR):
            t_tile = tpool.tile([128, OW * B], F32)
            nc.gpsimd.memset(t_tile[96:97, :], 1.0)
            for tb in range(8):
                ow0 = 8 * tb
                now = 8 if tb < 7 else 6
                npart = now * B
                pa = papool.tile([128, KCH], F32)
                for kw in range(KW):
                    src = xs[:, r, ow0 + kw : ow0 + kw + now, :]
                    src = src.rearrange("b ow c -> ow b c")
                    nc.sync.dma_start(pa[:npart, kw * C : (kw + 1) * C], src)
                tp = tppool.tile([KCH, 128], F32)
                nc.tensor.transpose(
                    tp[:, :npart], pa[:npart, :KCH], identity[:npart, :npart]
                )
                nc.vector.tensor_copy(
                    t_tile[0:KCH, 128 * tb : 128 * tb + npart], tp[:, :npart]
                )
            T.append(t_tile)

        # --- main loop: 4 positions run concurrently in distinct PE column
        # groups (tile_position=(0,32j)); psum rows 32j..32j+16 hold pos 4g+j.
        for oh in range(RPC):
            stage = stpool.tile([128, 16, F], F32)
            for kg in range(8):
                p0 = oh * OW + kg * 8
                np_ = 8 if kg < 7 else 6
                kt = ktpool.tile([128, 8, KH, F], F32)
                nc.sync.dma_start(kt[0:KCH, :, :, :], ks2[oh * 8 + kg])
                nc.sync.dma_start(
                    kt[96:97, 0:np_, 0, :], bs[None, p0 : p0 + np_, :]
                )
                for sub in range(2):
                    g = kg * 2 + sub
                    cnt = min(4, np_ - sub * 4)
                    if cnt <= 0:
                        continue
                    ps = pspool.tile([128, F], F32)
                    # chunk-major waves: all col groups issue chunk ch
                    # back-to-back so the 4 groups overlap in the array
                    for ch in range(KH):
                        kp = 97 if ch == 0 else KCH
                        for j in range(cnt):
                            ow = g * 4 + j
                            i = sub * 4 + j
                            fs = ow * B
                            nc.tensor.matmul(
                                ps[32 * j : 32 * j + B, :],
                                T[oh + ch][0:kp, fs : fs + B],
                                kt[0:kp, i, ch, :],
                                start=(ch == 0),
                                stop=(ch == KH - 1),
                                tile_position=(0, 32 * j),
                            )
                    if cnt == 4:
                        nc.vector.tensor_copy(stage[:, g, :], ps[:, :])
                    else:
                        for j in range(cnt):
                            nc.vector.tensor_copy(
                                stage[32 * j : 32 * j + B, g, :],
                                ps[32 * j : 32 * j + B, :],
                            )
            # out[b, oh, 4g+j, f] = stage[32j+b, g, f]
            full = out[:, oh, 0 : 4 * 15, :].rearrange("b (g j) f -> b g j f", j=4)
            for j in range(4):
                nc.sync.dma_start(full[:, :, j, :], stage[32 * j : 32 * j + B, 0:15, :])
            for j in range(2):
                nc.sync.dma_start(
                    out[:, oh, 60 + j, :], stage[32 * j : 32 * j + B, 15, :]
                )

    nc.finalize()
    _cached["nc"] = nc
    return nc


def _shard_inputs(x, kernel, bias):
    x = np.ascontiguousarray(np.asarray(x, dtype=np.float32))
    kernel = np.ascontiguousarray(np.asarray(kernel, dtype=np.float32))
    bias = np.ascontiguousarray(np.asarray(bias, dtype=np.float32))
    in_maps = []
    for c in range(NCORES):
        r0 = RPC * c
        nrows = min(NXR, H - r0)
        xs_c = np.zeros((B, NXR, W, C), dtype=np.float32)
        xs_c[:, :nrows] = x[:, r0 : r0 + nrows]
        p0 = PPC * c
        pe = min(p0 + PPC, OH * OW)
        ks_c = np.zeros((PPC, KSZ, F), dtype=np.float32)
        ks_c[: pe - p0] = kernel[p0:pe]
        bs_c = np.zeros((PPC, F), dtype=np.float32)
        bs_c[: pe - p0] = bias[p0:pe]
        # pre-transpose kernel shard into the SBUF tile layout (see ks2 decl)
        ks2_c = np.zeros((RPC * 8, KCH, 8, KH, F), dtype=np.float32)
        for oh in range(RPC):
            for kg in range(8):
                n = 8 if kg < 7 else 6
                blk = ks_c[oh * OW + kg * 8 : oh * OW + kg * 8 + n]  # (n, 288, 64)
                ks2_c[oh * 8 + kg, :, :n] = blk.reshape(n, KH, KCH, F).transpose(
                    2, 0, 1, 3
                )
        in_maps.append({"xs": xs_c, "ks2": ks2_c, "bs": bs_c})
    return in_maps


def _run(x, kernel, bias, trace=False):
    nc = _build_program()
    in_maps = _shard_inputs(x, kernel, bias)
    res = run_bass_kernel_spmd(nc, in_maps, core_ids=list(range(NCORES)), trace=trace)
    out_full = np.empty((B, OH, OW, F), dtype=np.float32)
    for c in range(NCORES):
        rows = min(RPC, OH - RPC * c)
        out_full[:, RPC * c : RPC * c + rows] = res.results[c]["out"][:, :rows]
    return out_full, res


def kernel(x, kernel, bias):
    out, _ = _run(x, kernel, bias, trace=False)
    return out
